# revision 25
# baseline (speedup 1.0000x reference)
"""Trainium2 Bass kernel for nn_EncoderMemNN_14929306321427 (MemNN encoder).

Math (see reference.py): story (M=256, B=16, S=64) token ids; C (4, V, 128)
embedding tables. Per hop h: m_A = sum_S C[h][s], prob = softmax_M(m_A @ u),
m_C = sum_S C[h+1][s], u += prob @ m_C. u starts at 0, so hop-0's softmax is
uniform: C[0] is never needed and u after hop 0 is mean_M(E1).

Strategy: data-parallel over batch (2 rows/core, 8 cores, no collectives).
Host fuses tables 1..3 into ccat[V+2, 384] fp16 so each token is ONE 768B
dma_gather row. dma_gather indices are int16; the vocab (50257) exceeds the
int16 range, so indices are stored BIASED by 17491 and the gather reads from
a +17491-row base: the ucode sign-extends negative idx into addresses
(verified on HW), covering the whole vocab in a single call with zero filler.
Only TRAILING negative indices are skipped by the ucode, so each call carries
one extra guard descriptor (idx 32767 -> an all-zero row past the vocab)
whose landing slot is never consumed.

Gathers are split into ~13-slot chunks issued round-robin across 4 SWDGE
queues so the DMA engines stay continuously fed; the PE consumes each chunk
as it lands (identity-matmul accumulation into PSUM, fp32-exact). A tiny
priming gather per queue absorbs the ~20us cold-start of the gather ucode
during the preamble. The attention tail keeps all softmax state slot-major
(logits computed transposed via F = E^T), which needs only a handful of tiny
matmuls: exp -> mask -> den (ones-matmul) -> weighted sum. The hop-0 state
u1 = mean_M(E1) is accumulated per group in row- and column-major in
parallel (separate PSUM banks - one open accumulation chain per bank).
"""

import numpy as np

HOPS = 3
V = 50257
D = 128
M = 256
B = 16
S = 64
NCORES = 8
BL = B // NCORES            # batch rows per core
NS = BL * M                 # sentences per core
P = 128
NG = NS // P                # sentence groups of 128
DCAT = HOPS * D             # 384 = fused row [C1|C2|C3]
BIAS = 17491                # idx bias: token t -> int16 t-BIAS
GUARD = 32767               # guard idx -> row BIAS+32767 = V+1 (all zero)
CH = 13                     # max gather-chunk slots
NQ = 4                      # SWDGE queues
GB = 14                     # gather chunk buffers in flight
LAG = 7                     # PE starts LAG chunks behind the gather stream
SP = False                  # dma_gather single_packet
SCRATCH = 49152             # SWDGE descriptor carveout bytes
DVECH = 0                   # last-group chunks accumulated on DVE (0 = off)

# const blob column offsets (f32, [128, BLOBC])
OFF_IDENT = 0
OFF_SEL = 128               # sel[p, g*2+b] = 1 if group g owned by b
OFF_I2 = 136                # 2x2 identity in rows 0..1
OFF_ONES = 138              # all-ones column
OFF_SEL2 = 139              # sel2[g*2+b, b] = 1 in rows 0..7
BLOBC = 141

_CACHE = {}


def _chunk_plan(g):
    """Split the S=64 slots into chunks of at most CH. Group 0 leads with
    one small chunk per queue so the per-queue cold desc-gen cost lands on
    small calls while the preamble/idx uploads still run."""
    if g == 0:
        sizes = [4] * 4 + [12] * 4
    else:
        n = -(-S // CH)
        base, rem = divmod(S, n)
        sizes = [base + (1 if i < rem else 0) for i in range(n)]
    assert sum(sizes) == S
    out, off = [], 0
    for s in sizes:
        out.append((off, s))
        off += s
    return out


def build(do_compile=True):
    from concourse import bacc, mybir, tile

    f32 = mybir.dt.float32
    f16 = mybir.dt.float16
    i16 = mybir.dt.int16
    Alu = mybir.AluOpType
    Act = mybir.ActivationFunctionType

    idx_cols = [sum(8 * csz + 1 for _, csz in _chunk_plan(g))
                for g in range(NG)]

    nc = bacc.Bacc(num_swdge_queues=NQ, dynamic_dma_scratch_size=SCRATCH)
    ccat_d = nc.declare_dram_parameter("ccat", [V + 2, DCAT], f16, isOutput=False)
    idx_d = [
        nc.declare_dram_parameter(f"idx{g}", [P, idx_cols[g]], i16, isOutput=False)
        for g in range(NG)
    ]
    blob_d = nc.declare_dram_parameter("blob", [P, BLOBC], f32, isOutput=False)
    identg_d = nc.declare_dram_parameter("identg", [P, P], f16, isOutput=False)
    out_d = nc.declare_dram_parameter("out", [BL, D], f32, isOutput=True)

    with tile.TileContext(nc) as tc:
        with (
            tc.tile_pool(name="const", bufs=1) as cpool,
            tc.tile_pool(name="gather", bufs=GB) as gpool,
            tc.tile_pool(name="work", bufs=2) as wpool,
            tc.tile_pool(name="ps_e", bufs=2, space="PSUM") as ps_e,
            tc.tile_pool(name="ps_t", bufs=2, space="PSUM") as ps_t,
            tc.tile_pool(name="ps_us", bufs=1, space="PSUM") as ps_us,
            tc.tile_pool(name="ps_at", bufs=2, space="PSUM") as ps_at_pool,
        ):
            # priming gathers: the first dma_gather on real HW stalls the
            # Pool engine ~20us (cold ucode) with all queue drains blocked
            # behind it; absorb that during the preamble with tiny gathers
            # (indices memset to 0 = row BIAS, discarded; no uploads needed)
            pidx = cpool.tile([P, 8], i16)
            nc.vector.memset(pidx[:], 0)
            for q in range(NQ):
                pg = cpool.tile([P, 1, DCAT], f16, tag=f"prime{q}")
                nc.gpsimd.dma_gather(
                    out_ap=pg[:], in_ap=ccat_d[BIAS:, :], idxs_ap=pidx[:],
                    num_idxs=P, num_idxs_reg=P, elem_size=DCAT,
                    single_packet=SP, queue_num=q,
                )
            idx_sb = []
            for g in range(NG):
                t = cpool.tile([P, idx_cols[g]], i16, tag=f"idx{g}")
                nc.sync.dma_start(out=t[:], in_=idx_d[g][:])
                idx_sb.append(t)
            blob = cpool.tile([P, BLOBC], f32)
            nc.scalar.dma_start(out=blob[:], in_=blob_d[:])
            identg = cpool.tile([P, P], f16)
            nc.scalar.dma_start(out=identg[:], in_=identg_d[:])

            ident = blob[:, OFF_IDENT:OFF_IDENT + P]
            sel = blob[:, OFF_SEL:OFF_SEL + NG * BL]
            i2 = blob[0:BL, OFF_I2:OFF_I2 + BL]
            ones = blob[:, OFF_ONES:OFF_ONES + 1]
            sel2 = blob[0:NG * BL, OFF_SEL2:OFF_SEL2 + BL]

            # ---- gather + sentence-sum: E_all[p, g*DCAT+d] = sum_S ccat[tok]
            # plus per group: F1/F2 = E1^T/E2^T and u1 accumulation both ways
            # per-group layout [E1 | E2 | 1 | E3 | 1] (stride DCAT+2): the
            # ones columns ride along in the o-matmuls so the softmax
            # denominator accumulates in the same PSUM chain as o
            E_all = cpool.tile([P, NG * (DCAT + 2)], f32)
            for g in range(NG):
                nc.vector.memset(E_all[:, g * (DCAT + 2) + 256:
                                       g * (DCAT + 2) + 257], 1.0)
                nc.vector.memset(E_all[:, g * (DCAT + 2) + 385:
                                       g * (DCAT + 2) + 386], 1.0)
            F1 = cpool.tile([P, NS], f32)
            F2 = cpool.tile([P, NS], f32)
            usbk = ps_us.tile([P, 512], f32, tag="us")
            ustbk = ps_us.tile([P, 512], f32, tag="ust")
            us_ps = usbk[0:BL, 0:D]
            ust_ps = ustbk[:, 0:BL]
            # gathers are emitted LAG chunks ahead of the PE work that
            # consumes them: a dummy matmul reading chunk LAG's tile delays
            # the PE's first real matmul until a backlog exists, so the PE
            # then runs continuously (high p-state) and per-chunk DMA
            # completion latency is hidden
            all_chunks = []
            for g in range(NG):
                ccol = 0
                for off, csz in _chunk_plan(g):
                    all_chunks.append((g, ccol, csz))
                    ccol += 8 * csz + 1
            nchunks = len(all_chunks)
            last_of_group = {}
            ord_in_group = []
            _seen = {}
            for i, (g, _, _) in enumerate(all_chunks):
                last_of_group[g] = i
                ord_in_group.append(_seen.get(g, 0))
                _seen[g] = _seen.get(g, 0) + 1
            tiles = [None] * nchunks
            eps_t = {}
            nmm_g = {g: 0 for g in range(NG)}
            # the last group's first DVECH chunks accumulate on the (idle)
            # Vector engine instead of the PE, trimming the PE's
            # end-of-stream backlog; merged into E_all with the copy pass
            dve_slots = sum(
                csz for (g, _, csz), o in zip(all_chunks, ord_in_group)
                if g == NG - 1 and o < DVECH)
            pe_slots = {g: S for g in range(NG)}
            pe_slots[NG - 1] = S - dve_slots
            acc = wpool.tile([P, DCAT], f32, tag="dveacc")
            ndve = [0]

            def emit_gather(i):
                g, ccol, csz = all_chunks[i]
                gt = gpool.tile([P, CH + 1, DCAT], f16, tag="ch")
                tiles[i] = gt
                nc.gpsimd.dma_gather(
                    out_ap=gt[:, :csz + 1, :], in_ap=ccat_d[BIAS:, :],
                    idxs_ap=idx_sb[g][:, ccol:ccol + 8 * csz + 1],
                    num_idxs=P * csz + 1, num_idxs_reg=P * csz + 1,
                    elem_size=DCAT, single_packet=SP,
                    queue_num=i % NQ,
                )

            def emit_pe(i):
                g, _, csz = all_chunks[i]
                if g not in eps_t:
                    eps_t[g] = ps_e.tile([P, DCAT], f32, tag="eacc",
                                         name=f"eps{g}")
                eps = eps_t[g]
                gt = tiles[i]
                if g == NG - 1 and ord_in_group[i] < DVECH:
                    for r in range(csz):
                        if ndve[0] == 0:
                            nc.vector.tensor_copy(out=acc[:], in_=gt[:, r, :])
                        else:
                            nc.vector.tensor_tensor(
                                out=acc[:], in0=acc[:], in1=gt[:, r, :],
                                op=Alu.add)
                        ndve[0] += 1
                else:
                    for r in range(csz):
                        nc.tensor.matmul(
                            out=eps[:], lhsT=identg[:], rhs=gt[:, r, :],
                            start=(nmm_g[g] == 0),
                            stop=(nmm_g[g] == pe_slots[g] - 1),
                        )
                        nmm_g[g] += 1
                if i != last_of_group[g]:
                    return
                gb0 = g * (DCAT + 2)
                if g == NG - 1 and DVECH > 0:
                    nc.vector.scalar_tensor_tensor(
                        out=E_all[:, gb0:gb0 + 256], in0=eps[:, 0:256],
                        scalar=1.0, in1=acc[:, 0:256],
                        op0=Alu.mult, op1=Alu.add)
                    nc.vector.scalar_tensor_tensor(
                        out=E_all[:, gb0 + 257:gb0 + 385], in0=eps[:, 256:384],
                        scalar=1.0, in1=acc[:, 256:384],
                        op0=Alu.mult, op1=Alu.add)
                else:
                    nc.vector.tensor_copy(out=E_all[:, gb0:gb0 + 256], in_=eps[:, 0:256])
                    nc.vector.tensor_copy(
                        out=E_all[:, gb0 + 257:gb0 + 385], in_=eps[:, 256:384])
                for t, F in ((0, F1), (1, F2)):
                    tp = ps_t.tile([P, P], f32, tag="tp")
                    nc.tensor.transpose(
                        out=tp[:],
                        in_=E_all[:, gb0 + t * D: gb0 + t * D + D],
                        identity=ident,
                    )
                    nc.scalar.copy(out=F[:, g * P:(g + 1) * P], in_=tp[:])
                # hop 0: u1 = mean_M E1 (softmax of zero logits is uniform),
                # accumulated row-major (us) and col-major (ust) in parallel
                nc.tensor.matmul(
                    out=us_ps[:], lhsT=sel[:, 2 * g:2 * g + 2],
                    rhs=E_all[:, gb0:gb0 + D],
                    start=(g == 0), stop=(g == NG - 1),
                )
                nc.tensor.matmul(
                    out=ust_ps[:], lhsT=E_all[:, gb0:gb0 + D],
                    rhs=sel[:, 2 * g:2 * g + 2],
                    start=(g == 0), stop=(g == NG - 1),
                )

            for i in range(nchunks):
                emit_gather(i)
                if i == LAG:
                    dly = ps_t.tile([P, P], f32, tag="tp")
                    nc.tensor.matmul(out=dly[:], lhsT=identg[:],
                                     rhs=tiles[LAG][:, 0, 0:P],
                                     start=True, stop=True)
                if i >= LAG:
                    emit_pe(i - LAG)
            for i in range(max(0, nchunks - LAG), nchunks):
                emit_pe(i)

            u = wpool.tile([BL, D], f32, tag="u0")
            nc.scalar.activation(out=u[:], in_=us_ps[:], func=Act.Copy, scale=1.0 / M)
            uc = wpool.tile([P, BL], f32, tag="uc0")
            nc.scalar.activation(out=uc[:], in_=ust_ps[:], func=Act.Copy, scale=1.0 / M)

            # ---- hops 1..2, all slot-major: logits lgT[slot, g*2+b]
            for hop in (1, 2):
                F = F1 if hop == 1 else F2
                # per-hop PSUM bank for logits + o; tiny tiles live in the
                # retired u1 banks (each bank has one open chain at a time)
                at = ps_at_pool.tile([P, 512], f32, tag="at")
                lgT = at[:, 0:NG * BL]
                o_ps = at[0:BL, 16:16 + D + 1]
                ucps = ustbk[:, 200 + 10 * hop:202 + 10 * hop]
                for g in range(NG):
                    nc.tensor.matmul(
                        out=lgT[:, 2 * g:2 * g + 2], lhsT=F[:, g * P:(g + 1) * P],
                        rhs=uc[:], start=True, stop=True,
                    )
                # exp (no max-sub: |logits| <~ 6 by construction), then mask
                # to the owning batch row (sel is exactly the 0/1 ownership)
                pe_raw = wpool.tile([P, NG * BL], f32, tag="praw")
                nc.scalar.activation(out=pe_raw[:], in_=lgT, func=Act.Exp)
                pe_t = wpool.tile([P, NG * BL], f32, tag="pet")
                nc.vector.scalar_tensor_tensor(
                    out=pe_t[:], in0=pe_raw[:], scalar=1.0, in1=sel[:],
                    op0=Alu.mult, op1=Alu.mult,
                )
                # o[b, 0:D] = sum_slots pe_t[slot, b] * E_{hop+1}[slot, :];
                # col D rides the ones column = softmax denominator
                rhs0 = 128 if hop == 1 else 257
                for g in range(NG):
                    nc.tensor.matmul(
                        out=o_ps, lhsT=pe_t[:, 2 * g:2 * g + 2],
                        rhs=E_all[:, g * (DCAT + 2) + rhs0:
                                  g * (DCAT + 2) + rhs0 + D + 1],
                        start=(g == 0), stop=(g == NG - 1),
                    )
                rden = wpool.tile([BL, 1], f32, tag="rden")
                nc.vector.reciprocal(out=rden[:], in_=o_ps[0:BL, D:D + 1])
                u2 = wpool.tile([BL, D], f32, tag=f"u{hop}")
                nc.vector.scalar_tensor_tensor(
                    out=u2[:], in0=o_ps[0:BL, 0:D], scalar=rden[:], in1=u[:],
                    op0=Alu.mult, op1=Alu.add,
                )
                u = u2
                if hop < HOPS - 1:
                    nc.tensor.matmul(out=ucps, lhsT=u[:], rhs=i2[:],
                                     start=True, stop=True)
                    uc = wpool.tile([P, BL], f32, tag=f"uc{hop}")
                    nc.scalar.copy(out=uc[:], in_=ucps)

            nc.sync.dma_start(out=out_d[:], in_=u[:])
    if do_compile:
        nc.compile()
    return nc


def _wrap16(idx):
    """flat [n] int16 -> wrapped [16, ceil(n/16)] (value i at [i%16, i//16]),
    replicated to the 8 16-partition groups the Q7 cores read."""
    n = idx.shape[0]
    cols = -(-n // 16)
    w = np.zeros((16, cols), np.int16)
    w[np.arange(n) % 16, np.arange(n) // 16] = idx
    return np.tile(w, (8, 1))


def prep_inputs(story, C):
    """Host-side: fused fp16 table + biased-int16 index layouts per core."""
    story = np.asarray(story)
    C = np.asarray(C, dtype=np.float32)
    s = story.transpose(1, 0, 2).astype(np.int32)       # (B, M, S)
    ccat = np.zeros((V + 2, DCAT), np.float16)
    ccat[:V] = np.concatenate([C[1], C[2], C[3]], axis=1).astype(np.float16)

    blob = np.zeros((P, BLOBC), np.float32)
    blob[:, OFF_IDENT:OFF_IDENT + P] = np.eye(P, dtype=np.float32)
    ngb = NG // BL                                       # groups per batch row
    for g in range(NG):
        # groups in natural order: sentences g*128..g*128+127, owner = g//ngb
        blob[:, OFF_SEL + g * 2 + g // ngb] = 1.0
        blob[g * 2 + g // ngb, OFF_SEL2 + g // ngb] = 1.0
    blob[0:BL, OFF_I2:OFF_I2 + BL] = np.eye(BL, dtype=np.float32)
    blob[:, OFF_ONES] = 1.0
    identg = np.eye(P, dtype=np.float16)

    in_maps = []
    for i in range(NCORES):
        m = {"ccat": ccat, "identg": identg, "blob": blob}
        blk = s[i * BL:(i + 1) * BL].reshape(NS, S)      # (NS, S)
        toks = np.sort(blk, axis=1)                      # vocab-sorted per sentence
        for g in range(NG):
            grp = toks[g * P:(g + 1) * P]                # (P, S)
            parts = []
            for off, csz in _chunk_plan(g):
                flat = (grp[:, off:off + csz].T.astype(np.int32) - BIAS)
                flat = flat.reshape(-1).astype(np.int16)
                parts.append(_wrap16(np.concatenate(
                    [flat, np.array([GUARD], np.int16)])))
            m[f"idx{g}"] = np.concatenate(parts, axis=1)
        in_maps.append(m)
    return in_maps


def run(in_maps, trace=False, **kwargs):
    from concourse.bass_utils import run_bass_kernel_spmd

    key = (CH, NQ, GB, SP, SCRATCH)
    if key not in _CACHE:
        _CACHE[key] = build()
    nc = _CACHE[key]
    res = run_bass_kernel_spmd(
        nc, in_maps, core_ids=list(range(NCORES)), trace=trace, **kwargs
    )
    out = np.concatenate([r["out"] for r in res.results], axis=0)
    return out, res


def kernel(story, C):
    in_maps = prep_inputs(story, C)
    out, _ = run(in_maps)
    return out.astype(np.float32)
